# revision 1
# baseline (speedup 1.0000x reference)
"""Preisach hysteresis (nn_BaseHysteresis) Bass kernel for 8 TRN2 cores.

Math: the per-relay state update is affine in the transformed state
shat = (s+1)/2:
    rising  (h > h_prev): shat' = g*shat + (1-g),  g = sigmoid(100*(alpha-h))
    falling (h < h_prev): shat' = g*shat,          g = sigmoid(100*(h-beta))
    equal              : shat' = shat              (g = 1, c = 0)
so per step: shat' = g*shat + c with
    g = sigmoid(arg_g), arg_g = 100*(alpha-h) rising / 100*(h-beta) falling /
                                +BIG on equal steps
    c = sigmoid(arg_c), arg_c = 100*(h-alpha) on rising steps, -BIG otherwise
Both args are rank-3 bilinear forms of per-relay params and per-step rows,
built by the tensor engine as [3,128]^T @ [3,L] float32r matmuls; ScalarE
applies sigmoid from PSUM; one DVE tensor_tensor_scan runs the entire
2048-step recurrence for 128 relays at a time; a dens-weighted matmul
reduces over relays into PSUM accumulators (lagged 3 blocks so the tensor
engine never stalls on a scan). The mesh dim M=20100 is sharded over 8
cores; the host sums the 8 partial reductions and applies the affine output.

Implementation is raw Bass (not Tile): the scan/activation ISA encodings on
this toolchain allow at most 0/1 sync waits per instruction, so all
cross-engine waits are emitted as standalone wait_ge instructions with
hand-computed semaphore thresholds.
"""

import os
from contextlib import ExitStack

import numpy as np

import concourse.bass as bass
import concourse.mybir as mybir
from concourse.bass_utils import run_bass_kernel_spmd

F32 = mybir.dt.float32
F32R = mybir.dt.float32r
BF16 = mybir.dt.bfloat16

L = 2048            # field sequence length
P = 128             # SBUF partitions
CHUNK = 512         # PSUM bank free size (f32)
HALF = 1024
NCHUNK = L // CHUNK
NBLK = 20           # relay blocks per core
RCORE = NBLK * P    # relays per core (2560)
NCORES = 8
CAP = RCORE * NCORES  # padded mesh size 20480
M = 20100
BIG = 10000.0
LAG = 3             # dens-reduce runs this many blocks behind the scans
NS = LAG + 1        # state-tile ring depth

USE_F32R = os.environ.get("KERNEL_F32R", "1") == "1"
MMDT = F32R if USE_F32R else F32

_last_results = None  # BassKernelResults of the most recent run (for test.py)


def _pe_base(b):
    """PE sem value before block b's arg matmuls: 8 args per earlier block
    plus 4 dens matmuls for each block emitted at lag LAG."""
    return 8 * b + 4 * max(0, b - LAG)


def build_program() -> bass.Bass:
    nc = bass.Bass("TRN2", target_bir_lowering=False)

    # f32r is bit-identical to f32 in memory; declaring the DRAM side as
    # f32r lets the matmuls consume the DMA'd tiles directly (no cast pass)
    xg_d = nc.dram_tensor("xg", [3, L], MMDT, kind="ExternalInput")
    xc_d = nc.dram_tensor("xc", [3, L], MMDT, kind="ExternalInput")
    wg_d = nc.dram_tensor("wg", [3, RCORE], MMDT, kind="ExternalInput")
    wc_d = nc.dram_tensor("wc", [3, RCORE], MMDT, kind="ExternalInput")
    dens_d = nc.dram_tensor("dens", [P, NBLK], F32, kind="ExternalInput")
    s0h_d = nc.dram_tensor("s0h", [P, NBLK], F32, kind="ExternalInput")
    out_d = nc.dram_tensor("partial", [1, L], F32, kind="ExternalOutput")

    sig = mybir.ActivationFunctionType.Sigmoid
    mult = mybir.AluOpType.mult
    add = mybir.AluOpType.add

    # act order per block: gA, gB, cA, cB (all 1024-wide)
    n_acts = 4 * NBLK
    pe_total = 8 * NBLK + 4 * NBLK  # 160 args + 80 dens

    with ExitStack() as ctx:
        xg_sb = ctx.enter_context(nc.sbuf_tensor([3, L], MMDT))
        xc_sb = ctx.enter_context(nc.sbuf_tensor([3, L], MMDT))
        wg_sb = ctx.enter_context(nc.sbuf_tensor([3, RCORE], MMDT))
        wc_sb = ctx.enter_context(nc.sbuf_tensor([3, RCORE], MMDT))
        dens_raw = ctx.enter_context(nc.sbuf_tensor([P, NBLK], F32))
        s0h_sb = ctx.enter_context(nc.sbuf_tensor([P, NBLK], F32))
        dens_sb = ctx.enter_context(nc.sbuf_tensor([P, NBLK], BF16))
        warm = ctx.enter_context(nc.sbuf_tensor([3, CHUNK], BF16))
        g0 = ctx.enter_context(nc.sbuf_tensor([P, L], F32))
        g1 = ctx.enter_context(nc.sbuf_tensor([P, L], F32))
        c0 = ctx.enter_context(nc.sbuf_tensor([P, L], F32))
        c1 = ctx.enter_context(nc.sbuf_tensor([P, L], F32))
        S = [ctx.enter_context(nc.sbuf_tensor(f"s{i}", [P, L], BF16))
             for i in range(NS)]
        pga = ctx.enter_context(nc.psum_tensor([P, HALF], F32))
        pgb = ctx.enter_context(nc.psum_tensor([P, HALF], F32))
        out_sb = ctx.enter_context(nc.sbuf_tensor([1, L], F32))
        acc = [ctx.enter_context(nc.psum_tensor(f"acc{k}", [1, CHUNK], F32))
               for k in range(NCHUNK)]
        s_dma = ctx.enter_context(nc.semaphore("s_dma"))
        s_prep = ctx.enter_context(nc.semaphore("s_prep"))
        s_pe = ctx.enter_context(nc.semaphore("s_pe"))
        s_act = ctx.enter_context(nc.semaphore("s_act"))
        s_dve = ctx.enter_context(nc.semaphore("s_dve"))
        s_warm = ctx.enter_context(nc.semaphore("s_warm"))
        block = ctx.enter_context(nc.Block())

        G = [g0, g1]
        C = [c0, c1]
        PG = [pga, pgb]

        def acc_ap(k):
            return acc[k][:, :]

        @block.sync
        def _(sync):
            for dst, src in ((xg_sb, xg_d), (xc_sb, xc_d), (wg_sb, wg_d),
                             (wc_sb, wc_d), (dens_raw, dens_d),
                             (s0h_sb, s0h_d)):
                sync.dma_start(dst[:, :], src[:, :]).then_inc(s_dma, 16)
            sync.wait_ge(s_act, n_acts + NCHUNK)
            sync.dma_start(out_d[:, :], out_sb[:, :]).then_inc(s_dma, 16)

        @block.gpsimd
        def _(gpsimd):
            gpsimd.memset(warm[:, :], 0.0).then_inc(s_warm, 1)

        @block.vector
        def _(vector):
            vector.wait_ge(s_dma, 6 * 16)
            vector.tensor_copy(dens_sb[:, :], dens_raw[:, :]
                               ).then_inc(s_prep, 1)
            for b in range(NBLK):
                vector.wait_ge(s_act, 4 * b + 4)   # all acts of block b
                if b >= NS:
                    # S[b%NS] was read by block (b-NS)'s dens matmuls,
                    # which are emitted in PE round b-NS+LAG = b-1
                    vector.wait_ge(s_pe, _pe_base(b - 1) + 12)
                vector.tensor_tensor_scan(
                    S[b % NS][:, :], G[b % 2][:, :], C[b % 2][:, :],
                    s0h_sb[:, b:b + 1], op0=mult, op1=add,
                ).then_inc(s_dve, 1)

        @block.tensor
        def _(tensor):
            # warm up the PE p-state during the DMA prologue (reads a
            # zeroed scratch tile; result discarded)
            tensor.wait_ge(s_warm, 1)
            for _ in range(10):
                tensor.matmul(pga[:, 0:CHUNK], warm[:, 0:P], warm[:, :],
                              start=True, stop=True)
            tensor.wait_ge(s_dma, 4 * 16)   # xg, xc, wg, wc loaded
            dens_ready = False

            def emit_dens(j):
                nonlocal dens_ready
                if not dens_ready:
                    tensor.wait_ge(s_prep, 1)
                    dens_ready = True
                tensor.wait_ge(s_dve, j + 1)   # scan(j) done
                for k in range(NCHUNK):
                    sl = slice(k * CHUNK, (k + 1) * CHUNK)
                    tensor.matmul(
                        acc_ap(k), dens_sb[:, j:j + 1], S[j % NS][:, sl],
                        start=(j == 0), stop=(j == NBLK - 1),
                        skip_group_check=True).then_inc(s_pe, 1)

            for b in range(NBLK):
                wgb = wg_sb[:, b * P:(b + 1) * P]
                wcb = wc_sb[:, b * P:(b + 1) * P]
                # g args: PG[half] was last read by the c-act of block b-1
                for half in range(2):
                    if b > 0:
                        tensor.wait_ge(s_act, 4 * (b - 1) + 3 + half)
                    for j in range(2):
                        lo = half * HALF + j * CHUNK
                        tensor.matmul(PG[half][:, j * CHUNK:(j + 1) * CHUNK],
                                      wgb, xg_sb[:, lo:lo + CHUNK],
                                      start=True, stop=True
                                      ).then_inc(s_pe, 1)
                # c args reuse PG[half] after the g-act of this block read it
                for half in range(2):
                    tensor.wait_ge(s_act, 4 * b + 1 + half)
                    for j in range(2):
                        lo = half * HALF + j * CHUNK
                        tensor.matmul(PG[half][:, j * CHUNK:(j + 1) * CHUNK],
                                      wcb, xc_sb[:, lo:lo + CHUNK],
                                      start=True, stop=True
                                      ).then_inc(s_pe, 1)
                if b >= LAG:
                    emit_dens(b - LAG)
            for j in range(NBLK - LAG, NBLK):
                emit_dens(j)

        @block.scalar
        def _(scalar):
            for b in range(NBLK):
                if b >= 2:
                    # G/C[b%2] were read by scan(b-2)
                    scalar.wait_ge(s_dve, b - 1)
                # act order: gA, gB, cA, cB
                for half in range(2):
                    hsl = slice(half * HALF, (half + 1) * HALF)
                    scalar.wait_ge(s_pe, _pe_base(b) + 2 * half + 2)
                    scalar.activation(G[b % 2][:, hsl], PG[half][:, :],
                                      sig).then_inc(s_act, 1)
                for half in range(2):
                    hsl = slice(half * HALF, (half + 1) * HALF)
                    scalar.wait_ge(s_pe, _pe_base(b) + 4 + 2 * half + 2)
                    scalar.activation(C[b % 2][:, hsl], PG[half][:, :],
                                      sig).then_inc(s_act, 1)
            scalar.wait_ge(s_pe, pe_total)
            for k in range(NCHUNK):
                sl = slice(k * CHUNK, (k + 1) * CHUNK)
                scalar.copy(out_sb[:, sl], acc_ap(k)).then_inc(s_act, 1)

    return nc


def make_core_inputs(x, mesh_points, raw_density, current_state, current_field,
                     h_min, h_range):
    """Host-side preprocessing: normalized field, step directions, padded
    per-core parameter tensors. Returns (in_maps, norm_h, dens_sum)."""
    f = np.float32
    x = np.asarray(x, f)
    h = ((x - f(h_min)) / f(h_range)).astype(f)
    hprev = np.empty_like(h)
    hprev[0] = f(current_field)
    hprev[1:] = h[:-1]
    mu = (h > hprev).astype(f)   # rising steps
    md = (h < hprev).astype(f)   # falling steps
    me = 1.0 - mu - md           # equal steps

    bias_g = (mu * (-100.0 * h) + md * (100.0 * h) + me * BIG).astype(f)
    bias_c = (mu * (100.0 * h) + (1.0 - mu) * (-BIG)).astype(f)
    xg_row = np.stack([mu, md, bias_g], axis=0).astype(f)        # [3, L]
    xc_row = np.stack([mu, np.zeros_like(mu), bias_c], axis=0).astype(f)

    mesh = np.asarray(mesh_points, f)
    alpha = np.full(CAP, 0.5, f)
    beta = np.full(CAP, 0.5, f)
    alpha[:M] = mesh[:, 1]
    beta[:M] = mesh[:, 0]

    raw = np.asarray(raw_density, f)
    dens_full = np.zeros(CAP, f)
    dens_full[:M] = np.logaddexp(raw, f(0.0)).astype(f)  # softplus
    dens_sum = np.sum(dens_full[:M], dtype=f)

    s0_full = np.zeros(CAP, f)
    s0_full[:M] = ((np.asarray(current_state, f) + f(1.0)) * f(0.5))

    in_maps = []
    for c in range(NCORES):
        sl = slice(c * RCORE, (c + 1) * RCORE)
        a_c, b_c = alpha[sl], beta[sl]
        wg = np.stack([100.0 * a_c, -100.0 * b_c, np.ones(RCORE, f)], 0)
        wc = np.stack([-100.0 * a_c, np.zeros(RCORE, f), np.ones(RCORE, f)], 0)
        in_maps.append({
            "xg": xg_row,
            "xc": xc_row,
            "wg": wg.astype(f),
            "wc": wc.astype(f),
            # [P, NBLK]: column b = relays b*128..b*128+127 of this core
            "dens": dens_full[sl].reshape(NBLK, P).T.copy(),
            "s0h": s0_full[sl].reshape(NBLK, P).T.copy(),
        })
    return in_maps, h, dens_sum


def kernel(x, mesh_points, raw_density, offset, scale, slope,
           current_state, current_field, h_min, h_range):
    global _last_results
    f = np.float32
    in_maps, h, dens_sum = make_core_inputs(
        x, mesh_points, raw_density, current_state, current_field,
        h_min, h_range)

    nc = build_program()
    trace = os.environ.get("KERNEL_TRACE", "0") == "1"
    res = run_bass_kernel_spmd(nc, in_maps, list(range(NCORES)), trace=trace)
    _last_results = res

    num = np.zeros(L, f)
    for r in res.results:
        num += r["partial"].reshape(L)
    m = (f(2.0) * num / dens_sum - f(1.0)).astype(f)

    scale = np.asarray(scale, f)
    offset = np.asarray(offset, f)
    slope = np.asarray(slope, f)
    return (scale * m + offset + h * slope).astype(f)



# revision 6
# speedup vs baseline: 1.1313x; 1.1313x over previous
"""Preisach hysteresis (nn_BaseHysteresis) Bass kernel for 8 TRN2 cores.

Math: the per-relay state update is affine in the transformed state
shat = (s+1)/2:
    rising  (h > h_prev): shat' = g*shat + (1-g),  g = sigmoid(100*(alpha-h))
    falling (h < h_prev): shat' = g*shat,          g = sigmoid(100*(h-beta))
    equal              : shat' = shat              (g = 1)
i.e. shat' = g*shat + c with c = mu*(1-g), mu = rising-step indicator.
Since mu is shared by ALL relays, the change of variable z = shat - mu
absorbs c entirely:
    z_t = g_t * (z_{t-1} + d_t),   d_t = mu_{t-1} - mu_t   (mu_{-1} := 0)
so the kernel only ever computes g = sigmoid(arg_g):
    arg_g = 100*(alpha-h) rising / 100*(h-beta) falling / +BIG on equal
a rank-3 bilinear form of per-relay params and per-step rows, built by the
tensor engine as [3,128]^T @ [3,L] float32r matmuls; ScalarE applies
sigmoid from PSUM (bf16 out); one DVE tensor_tensor_scan per 128-relay
block runs the entire 2048-step z-recurrence (op0=add with the shared
broadcast d tile, op1=mult with g); a dens-weighted matmul reduces over
relays into PSUM accumulators (lagged 3 blocks so the tensor engine never
stalls on a scan). The mesh dim M=20100 is sharded over 8 cores; the host
sums the 8 partial reductions, adds the mu_t*sum(dens) correction
(dens^T shat = dens^T z + mu*sum(dens)), and applies the affine output.

Implementation is raw Bass (not Tile): the scan/activation ISA encodings on
this toolchain allow at most 0/1 sync waits per instruction, so all
cross-engine waits are emitted as standalone wait_ge instructions with
hand-computed semaphore thresholds.
"""

import os
from contextlib import ExitStack

import ml_dtypes
import numpy as np

import concourse.bass as bass
import concourse.mybir as mybir
from concourse.bass_utils import run_bass_kernel_spmd

F32 = mybir.dt.float32
F32R = mybir.dt.float32r
BF16 = mybir.dt.bfloat16

L = 2048            # field sequence length
P = 128             # SBUF partitions
CHUNK = 512         # PSUM bank free size (f32)
HALF = 1024
NCHUNK = L // CHUNK
NBLK = 20           # relay blocks per core
RCORE = NBLK * P    # relays per core (2560)
NCORES = 8
CAP = RCORE * NCORES  # padded mesh size 20480
M = 20100
BIG = 10000.0
LAG = 3             # dens-reduce runs this many blocks behind the scans
NS = LAG + 1        # state-tile ring depth

_last_results = None  # BassKernelResults of the most recent run (for test.py)


def _pe_base(b):
    """PE sem value before block b's arg matmuls: 4 args per earlier block
    plus 4 dens matmuls for each block emitted at lag LAG."""
    return 4 * b + 4 * max(0, b - LAG)


def build_program() -> bass.Bass:
    nc = bass.Bass("TRN2", target_bir_lowering=False)

    # f32r is bit-identical to f32 in memory; declaring the DRAM side as
    # f32r lets the matmuls consume the DMA'd tiles directly (no cast pass)
    xg_d = nc.dram_tensor("xg", [3, L], F32R, kind="ExternalInput")
    wg_d = nc.dram_tensor("wg", [3, RCORE], F32R, kind="ExternalInput")
    s0h_d = nc.dram_tensor("s0h", [P, NBLK], F32, kind="ExternalInput")
    dbc_d = nc.dram_tensor("dbc", [P, L], BF16, kind="ExternalInput")
    dens_d = nc.dram_tensor("dens", [P, NBLK], F32, kind="ExternalInput")
    out_d = nc.dram_tensor("partial", [1, L], F32, kind="ExternalOutput")

    sig = mybir.ActivationFunctionType.Sigmoid
    mult = mybir.AluOpType.mult
    add = mybir.AluOpType.add

    # act order per block: gA, gB (each 1024-wide)
    n_acts = 2 * NBLK
    pe_total = 4 * NBLK + 4 * NBLK  # 80 args + 80 dens

    with ExitStack() as ctx:
        xg_sb = ctx.enter_context(nc.sbuf_tensor([3, L], F32R))
        wg_sb = ctx.enter_context(nc.sbuf_tensor([3, RCORE], F32R))
        s0h_sb = ctx.enter_context(nc.sbuf_tensor([P, NBLK], F32))
        dbc_sb = ctx.enter_context(nc.sbuf_tensor([P, L], BF16))
        dens_raw = ctx.enter_context(nc.sbuf_tensor([P, NBLK], F32))
        dens_sb = ctx.enter_context(nc.sbuf_tensor([P, NBLK], BF16))
        warm = ctx.enter_context(nc.sbuf_tensor([3, CHUNK], BF16))
        g0 = ctx.enter_context(nc.sbuf_tensor([P, L], BF16))
        g1 = ctx.enter_context(nc.sbuf_tensor([P, L], BF16))
        S = [ctx.enter_context(nc.sbuf_tensor(f"s{i}", [P, L], BF16))
             for i in range(NS)]
        pga = ctx.enter_context(nc.psum_tensor([P, HALF], F32))
        pgb = ctx.enter_context(nc.psum_tensor([P, HALF], F32))
        out_sb = ctx.enter_context(nc.sbuf_tensor([1, L], F32))
        acc = [ctx.enter_context(nc.psum_tensor(f"acc{k}", [1, CHUNK], F32))
               for k in range(NCHUNK)]
        s_dma = ctx.enter_context(nc.semaphore("s_dma"))
        s_dma_pe = ctx.enter_context(nc.semaphore("s_dma_pe"))
        s_dma_vec = ctx.enter_context(nc.semaphore("s_dma_vec"))
        s_prep = ctx.enter_context(nc.semaphore("s_prep"))
        s_pe = ctx.enter_context(nc.semaphore("s_pe"))
        s_act = ctx.enter_context(nc.semaphore("s_act"))
        s_dve = ctx.enter_context(nc.semaphore("s_dve"))
        s_warm = ctx.enter_context(nc.semaphore("s_warm"))
        block = ctx.enter_context(nc.Block())

        G = [g0, g1]
        PG = [pga, pgb]

        def acc_ap(k):
            return acc[k][:, :]

        @block.sync
        def _(sync):
            # per-consumer DMA semaphores: PE needs xg+wg; vector needs
            # s0h+dbc (scan 0) and dens (prep copy)
            sync.dma_start(xg_sb[:, :], xg_d[:, :]).then_inc(s_dma_pe, 16)
            sync.dma_start(wg_sb[:, :], wg_d[:, :]).then_inc(s_dma_pe, 16)
            sync.dma_start(s0h_sb[:, :], s0h_d[:, :]).then_inc(s_dma_vec, 16)
            sync.dma_start(dbc_sb[:, :], dbc_d[:, :]).then_inc(s_dma_vec, 16)
            sync.dma_start(dens_raw[:, :], dens_d[:, :]).then_inc(s_dma_vec, 16)
            sync.wait_ge(s_act, n_acts + NCHUNK)
            sync.dma_start(out_d[:, :], out_sb[:, :]).then_inc(s_dma, 16)

        @block.gpsimd
        def _(gpsimd):
            gpsimd.memset(warm[:, :], 0.0).then_inc(s_warm, 1)

        @block.vector
        def _(vector):
            vector.wait_ge(s_dma_vec, 3 * 16)
            vector.tensor_copy(dens_sb[:, :], dens_raw[:, :]
                               ).then_inc(s_prep, 1)
            for b in range(NBLK):
                vector.wait_ge(s_act, 2 * b + 2)   # both acts of block b
                if b >= NS:
                    # S[b%NS] was read by block (b-NS)'s dens matmuls,
                    # which are emitted in PE round b-NS+LAG = b-1
                    vector.wait_ge(s_pe, _pe_base(b - 1) + 8)
                vector.tensor_tensor_scan(
                    S[b % NS][:, :], dbc_sb[:, :], G[b % 2][:, :],
                    s0h_sb[:, b:b + 1], op0=add, op1=mult,
                ).then_inc(s_dve, 1)

        @block.tensor
        def _(tensor):
            # warm up the PE p-state during the DMA prologue (reads a
            # zeroed scratch tile; result discarded)
            tensor.wait_ge(s_warm, 1)
            for _ in range(10):
                tensor.matmul(pga[:, 0:CHUNK], warm[:, 0:P], warm[:, :],
                              start=True, stop=True)
            tensor.wait_ge(s_dma_pe, 2 * 16)   # xg, wg loaded
            dens_ready = False

            def emit_dens(j):
                nonlocal dens_ready
                if not dens_ready:
                    tensor.wait_ge(s_prep, 1)
                    dens_ready = True
                tensor.wait_ge(s_dve, j + 1)   # scan(j) done
                for k in range(NCHUNK):
                    sl = slice(k * CHUNK, (k + 1) * CHUNK)
                    tensor.matmul(
                        acc_ap(k), dens_sb[:, j:j + 1], S[j % NS][:, sl],
                        start=(j == 0), stop=(j == NBLK - 1),
                        skip_group_check=True).then_inc(s_pe, 1)

            for b in range(NBLK):
                wgb = wg_sb[:, b * P:(b + 1) * P]
                # PG[half] was last read by block b-1's act of that half
                for half in range(2):
                    if b > 0:
                        tensor.wait_ge(s_act, 2 * (b - 1) + half + 1)
                    for j in range(2):
                        lo = half * HALF + j * CHUNK
                        tensor.matmul(PG[half][:, j * CHUNK:(j + 1) * CHUNK],
                                      wgb, xg_sb[:, lo:lo + CHUNK],
                                      start=True, stop=True
                                      ).then_inc(s_pe, 1)
                if b >= LAG:
                    emit_dens(b - LAG)
            for j in range(NBLK - LAG, NBLK):
                emit_dens(j)

        @block.scalar
        def _(scalar):
            for b in range(NBLK):
                if b >= 2:
                    # G[b%2] was read by scan(b-2)
                    scalar.wait_ge(s_dve, b - 1)
                for half in range(2):
                    hsl = slice(half * HALF, (half + 1) * HALF)
                    scalar.wait_ge(s_pe, _pe_base(b) + 2 * (half + 1))
                    scalar.activation(G[b % 2][:, hsl], PG[half][:, :],
                                      sig).then_inc(s_act, 1)
            for k in range(NCHUNK):
                # dens matmuls of the last block fill acc chunks in order
                scalar.wait_ge(s_pe, pe_total - (NCHUNK - 1) + k)
                sl = slice(k * CHUNK, (k + 1) * CHUNK)
                scalar.copy(out_sb[:, sl], acc_ap(k)).then_inc(s_act, 1)

    return nc


def make_core_inputs(x, mesh_points, raw_density, current_state, current_field,
                     h_min, h_range):
    """Host-side preprocessing: normalized field, step directions, padded
    per-core parameter tensors. Returns (in_maps, norm_h, mu, dens_sum)."""
    f = np.float32
    x = np.asarray(x, f)
    h = ((x - f(h_min)) / f(h_range)).astype(f)
    hprev = np.empty_like(h)
    hprev[0] = f(current_field)
    hprev[1:] = h[:-1]
    mu = (h > hprev).astype(f)   # rising steps
    md = (h < hprev).astype(f)   # falling steps
    me = 1.0 - mu - md           # equal steps

    bias_g = (mu * (-100.0 * h) + md * (100.0 * h) + me * BIG).astype(f)
    xg_row = np.stack([mu, md, bias_g], axis=0).astype(f)        # [3, L]

    # z-scan additive input, shared by all relays: d_t = mu_{t-1} - mu_t
    d = np.empty(L, f)
    d[0] = -mu[0]
    d[1:] = mu[:-1] - mu[1:]
    dbc = np.broadcast_to(d, (P, L)).astype(ml_dtypes.bfloat16)  # exact

    mesh = np.asarray(mesh_points, f)
    alpha = np.full(CAP, 0.5, f)
    beta = np.full(CAP, 0.5, f)
    alpha[:M] = mesh[:, 1]
    beta[:M] = mesh[:, 0]

    raw = np.asarray(raw_density, f)
    dens_full = np.zeros(CAP, f)
    dens_full[:M] = np.logaddexp(raw, f(0.0)).astype(f)  # softplus
    dens_sum = np.sum(dens_full[:M], dtype=f)

    s0_full = np.zeros(CAP, f)
    s0_full[:M] = ((np.asarray(current_state, f) + f(1.0)) * f(0.5))

    in_maps = []
    for c in range(NCORES):
        sl = slice(c * RCORE, (c + 1) * RCORE)
        a_c, b_c = alpha[sl], beta[sl]
        wg = np.stack([100.0 * a_c, -100.0 * b_c, np.ones(RCORE, f)], 0)
        in_maps.append({
            "xg": xg_row,
            "wg": wg.astype(f),
            "s0h": s0_full[sl].reshape(NBLK, P).T.copy(),
            "dbc": dbc,
            # [P, NBLK]: column b = relays b*128..b*128+127 of this core
            "dens": dens_full[sl].reshape(NBLK, P).T.copy(),
        })
    return in_maps, h, mu, dens_sum


def kernel(x, mesh_points, raw_density, offset, scale, slope,
           current_state, current_field, h_min, h_range):
    global _last_results
    f = np.float32
    in_maps, h, mu, dens_sum = make_core_inputs(
        x, mesh_points, raw_density, current_state, current_field,
        h_min, h_range)

    nc = build_program()
    trace = os.environ.get("KERNEL_TRACE", "0") == "1"
    res = run_bass_kernel_spmd(nc, in_maps, list(range(NCORES)), trace=trace)
    _last_results = res

    num = np.zeros(L, f)
    for r in res.results:
        num += r["partial"].reshape(L)
    # dens^T shat = dens^T z + mu * sum(dens)
    m = (f(2.0) * (num / dens_sum + mu) - f(1.0)).astype(f)

    scale = np.asarray(scale, f)
    offset = np.asarray(offset, f)
    slope = np.asarray(slope, f)
    return (scale * m + offset + h * slope).astype(f)
